# revision 1
# baseline (speedup 1.0000x reference)
"""Deep Richardson-Lucy deconvolution on 8 Trainium2 NeuronCores.

Strategy (per core, data-parallel batch shard of 512 rows):
- Everything lives in SBUF in a TRANSPOSED fp16 layout: [l on partitions
  (64 blocks of 128), batch on the free dim (512)].
- conv1d(K=31, zero-pad) == banded-Toeplitz matmul per 128-l block:
  one center [128,128] matmul + two 32-row halo matmuls against the
  neighbor blocks, packed to array corners via tile_position.
- Per RL iteration: conv(s) -> PSUM; r = ACT.Reciprocal(psum + EPS);
  ratio = m * r (DVE fp16 2x); conv(ratio, flipped) -> PSUM;
  s *= psum (DVE, PSUM operand).
- In/out transposes ride the DMA xbar transpose engine (fp16).
"""
import hashlib
import numpy as np

EPS = 1e-6
P = 128
KTAPS = 31
PAD = 15
B_FULL, L = 4096, 8192
N_CORES = 8
BC = B_FULL // N_CORES          # 512 batch rows per core
NT = L // P                     # 64 l-blocks
NITER = 10

_cache = {}


def _build_toeplitz(psf):
    Wc = np.zeros((P, P), dtype=np.float64)
    j = np.arange(P)[:, None]
    i = np.arange(P)[None, :]
    k = j - i + PAD
    m = (k >= 0) & (k < KTAPS)
    Wc[m] = psf[k[m]]
    WL = np.zeros((32, 32), dtype=np.float64)   # rhs = prev block parts [96,128)
    jj = np.arange(32)[:, None]
    ii = np.arange(32)[None, :]
    k = (96 + jj - 128) - ii + PAD
    m = (k >= 0) & (k < KTAPS)
    WL[m] = psf[k[m]]
    WR = np.zeros((32, 32), dtype=np.float64)   # rhs = next block parts [0,32)
    k = (jj + 128) - (96 + ii) + PAD
    m = (k >= 0) & (k < KTAPS)
    WR[m] = psf[k[m]]
    return Wc, WL, WR


def _wpack(psf):
    Wc, WL, WR = _build_toeplitz(psf)
    w = np.zeros((P, 192), dtype=np.float16)
    w[:, 0:128] = Wc
    w[96:128, 128:160] = WL
    w[0:32, 160:192] = WR
    return w


def _r0pack(psf64):
    """r0[p, t] = 1 / (conv1d(0.5*ones, psf)[128t+p] + EPS)."""
    ones = np.full((1, L), 0.5, dtype=np.float64)
    xp = np.pad(ones, ((0, 0), (PAD, PAD)))
    sc = np.zeros((1, L), dtype=np.float64)
    for k in range(KTAPS):
        sc += xp[:, k:k + L] * psf64[k]
    r = 1.0 / (sc[0] + EPS)
    return r.reshape(NT, P).T.astype(np.float32)


def _build(psf64, alpha64):
    import concourse.bass as bass
    import concourse.tile as tile
    from concourse import mybir
    import bass_rust

    F32 = mybir.dt.float32
    F16 = mybir.dt.float16

    class SafeTC(tile.TileContext):
        # this walrus build rejects >1 sync wait per CTRL-class instruction
        def _drain_and_barrier(self, tick_clock, wait_clock):
            gc = tick_clock.global_clock
            for i in range(len(gc)):
                if gc[i] > 0:
                    di = self.nc.sync.drain()
                    pc = bass_rust.VectorClock()
                    pc.require_at_least(i, gc[i])
                    wait_clock.add_sem_waits(di.ins, bass_rust.ScopedClock({None: pc}))
            self.nc.all_engine_barrier()
            popped = self.nc._tile_sem_poison_stack.pop()
            assert popped is self._sem_poison
            self.nc.clear_and_free_semaphores(list(self.sems.allocated().values()))
            self.nc.all_engine_barrier()

    def split_multi_waits(nc, max_waits=1):
        n_fixed = 0
        uid = [0]
        for f in nc.m.functions:
            for bb in f.blocks:
                out = []
                changed = False
                for inst in bb.instructions:
                    si = inst.sync_info
                    if si is not None:
                        sems = [w for w in si.on_wait
                                if str(getattr(w, "sync_type", "")) == "semaphore"]
                        other = [w for w in si.on_wait if w not in sems]
                        if len(sems) > max_waits:
                            keep = sems[-max_waits:]
                            for w in sems[:-max_waits]:
                                nop = mybir.InstNoOp(
                                    name=f"waitsplit_{uid[0]}", ins=[], outs=[])
                                uid[0] += 1
                                nop.engine = inst.engine
                                nop.sync_info = mybir.SyncInfo(
                                    on_wait=[w], on_update=[])
                                out.append(nop)
                            inst.sync_info = mybir.SyncInfo(
                                on_wait=other + keep,
                                on_update=list(si.on_update))
                            n_fixed += 1
                            changed = True
                    out.append(inst)
                if changed:
                    try:
                        bb.instructions = out
                    except Exception:
                        bb.instructions.clear()
                        bb.instructions.extend(out)
        return n_fixed

    def act_raw(nc, out, in_, func, bias=0.0, scale=1.0):
        eng = nc.scalar
        ins = [eng.lower_ap(in_),
               mybir.ImmediateValue(dtype=F32, value=float(bias)),
               mybir.ImmediateValue(dtype=F32, value=float(scale)),
               mybir.ImmediateValue(dtype=F32, value=0.0)]
        return eng.add_instruction(mybir.InstActivation(
            name=nc.get_next_instruction_name(), func=func, ins=ins,
            outs=[eng.lower_ap(out)]))

    alpha_is_one = bool(np.all(alpha64 == 1.0))

    nc = bass.Bass("TRN2", target_bir_lowering=False, debug=False,
                   num_devices=N_CORES)
    m_in = nc.dram_tensor("m", [BC, L], F32, kind="ExternalInput")
    w1_in = nc.dram_tensor("w1", [P, 192], F16, kind="ExternalInput")
    w2_in = nc.dram_tensor("w2", [P, 192], F16, kind="ExternalInput")
    r0_in = nc.dram_tensor("r0", [P, NT], F32, kind="ExternalInput")
    y_out = nc.dram_tensor("y", [BC, L], F32, kind="ExternalOutput")

    Rec = mybir.ActivationFunctionType.Reciprocal
    Ln = mybir.ActivationFunctionType.Ln
    Exp = mybir.ActivationFunctionType.Exp

    def conv_block(psum, w, src, t, start_grp):
        last = "R" if t < NT - 1 else ("L" if t > 0 else "C")
        nc.tensor.matmul(psum[:], w[:, 0:128], src[:, t, :],
                         start=start_grp, stop=(last == "C"))
        if t > 0:
            nc.tensor.matmul(psum[0:32, :], w[96:128, 128:160],
                             src[96:128, t - 1, :], start=False,
                             stop=(last == "L"), tile_position=(96, 0))
        if t < NT - 1:
            nc.tensor.matmul(psum[96:128, :], w[0:32, 160:192],
                             src[0:32, t + 1, :], start=False,
                             stop=(last == "R"), tile_position=(0, 96))

    with SafeTC(nc) as tc:
        with tc.tile_pool(name="wpool", bufs=1) as wpool, \
             tc.tile_pool(name="mpool", bufs=1) as mpool, \
             tc.tile_pool(name="spool", bufs=1) as spool:
            w1 = wpool.tile([P, 192], F16)
            nc.sync.dma_start(w1[:], w1_in[:])
            w2 = wpool.tile([P, 192], F16)
            nc.sync.dma_start(w2[:], w2_in[:])
            r0 = wpool.tile([P, NT], F32)
            nc.sync.dma_start(r0[:], r0_in[:])
            mT = mpool.tile([P, NT, BC], F16)
            s = spool.tile([P, NT, BC], F16)
            nc.vector.memset(s[:], 0.5)

            # ---- load m, cast fp16, DMA-xbar transpose into mT ----
            with tc.tile_pool(name="stage", bufs=1) as stage:
                for c in range(BC // P):
                    st32 = stage.tile([P, L], F32, tag="st32")
                    nc.sync.dma_start(st32[:], m_in[c * P:(c + 1) * P, :])
                    st16 = stage.tile([P, L], F16, tag="st16")
                    nc.vector.tensor_copy(st16[:], st32[:])
                    nc.sync.dma_start_transpose(
                        mT[:, :, c * P:(c + 1) * P], st16[:])

            # ---- RL iterations ----
            with tc.tile_pool(name="ratio", bufs=8) as rpool, \
                 tc.tile_pool(name="rtile", bufs=4) as rtp, \
                 tc.tile_pool(name="psum", bufs=6, space="PSUM") as pp:
                for it in range(NITER):
                    ratio_tiles = [None] * NT

                    def _ratio(t):
                        ra = rpool.tile([P, BC], F16, tag="ra")
                        if it == 0:
                            # s == 0.5 everywhere: conv(s)+EPS is a per-l
                            # constant; r0 = 1/that, precomputed on host.
                            nc.vector.tensor_scalar(
                                out=ra[:], in0=mT[:, t, :],
                                scalar1=r0[:, t:t + 1], scalar2=None,
                                op0=mybir.AluOpType.mult)
                        else:
                            ps = pp.tile([P, BC], mybir.dt.float32, tag="ps")
                            conv_block(ps, w1, s, t, True)
                            rt = rtp.tile([P, BC], F16, tag="rt")
                            act_raw(nc, rt[:], ps[:], Rec, bias=EPS)
                            nc.vector.tensor_mul(ra[:], mT[:, t, :], rt[:])
                        ratio_tiles[t] = ra

                    def _conv2_update(t):
                        ps = pp.tile([P, BC], mybir.dt.float32, tag="ps")
                        last = "R" if t < NT - 1 else "L"
                        nc.tensor.matmul(ps[:], w2[:, 0:128],
                                         ratio_tiles[t][:], start=True,
                                         stop=False)
                        if t > 0:
                            nc.tensor.matmul(
                                ps[0:32, :], w2[96:128, 128:160],
                                ratio_tiles[t - 1][96:128, :], start=False,
                                stop=(last == "L"), tile_position=(96, 0))
                        if t < NT - 1:
                            nc.tensor.matmul(
                                ps[96:128, :], w2[0:32, 160:192],
                                ratio_tiles[t + 1][0:32, :], start=False,
                                stop=(last == "R"), tile_position=(0, 96))
                        if alpha_is_one:
                            if t % 2 == 0:
                                # DVE fused: s = (psum + EPS) * s, PSUM src 1x
                                nc.vector.scalar_tensor_tensor(
                                    out=s[:, t, :], in0=ps[:], scalar=EPS,
                                    in1=s[:, t, :],
                                    op0=mybir.AluOpType.add,
                                    op1=mybir.AluOpType.mult)
                            else:
                                # ACT evacuates PSUM (+EPS), DVE fp16 mul 2x
                                cp = rtp.tile([P, BC], F16, tag="cp")
                                act_raw(nc, cp[:], ps[:],
                                        mybir.ActivationFunctionType.Copy,
                                        bias=EPS)
                                nc.vector.tensor_mul(s[:, t, :], s[:, t, :],
                                                     cp[:])
                        else:
                            lg = rtp.tile([P, BC], F32, tag="lg")
                            act_raw(nc, lg[:], ps[:], Ln, bias=EPS)
                            cp = rtp.tile([P, BC], F16, tag="cp")
                            act_raw(nc, cp[:], lg[:], Exp,
                                    scale=float(alpha64[it]))
                            nc.vector.tensor_mul(s[:, t, :], s[:, t, :], cp[:])

                    # software-pipelined emission: keeps per-engine FIFO
                    # order producer/consumer-coupled so pool slots recycle
                    # without cross-engine ordering cycles.
                    _ratio(0)
                    _ratio(1)
                    for w in range(NT):
                        if w + 2 < NT:
                            _ratio(w + 2)
                        _conv2_update(w)

            # ---- transpose back + cast fp32 + store ----
            with tc.tile_pool(name="outp", bufs=1) as outp:
                for q in range(4):
                    sn16 = outp.tile([P, NT, P], F16, tag="sn16")
                    nc.sync.dma_start_transpose(sn16[:], s[:, q * 16:(q + 1) * 16, :])
                    sn32 = outp.tile([P, NT, P], F32, tag="sn32")
                    nc.vector.tensor_copy(sn32[:], sn16[:])
                    sn32r = sn32.rearrange("p (tl bc) lp -> p tl bc lp",
                                           tl=16, bc=4)
                    for bc in range(4):
                        ydst = y_out[bc * P:(bc + 1) * P,
                                     q * 2048:(q + 1) * 2048].rearrange(
                            "p (tl lp) -> p tl lp", lp=P)
                        nc.sync.dma_start(ydst, sn32r[:, :, bc, :])

    split_multi_waits(nc)
    return nc


def kernel(m, psf, alpha):
    m = np.asarray(m)
    psf64 = np.asarray(psf, dtype=np.float64)
    alpha64 = np.asarray(alpha, dtype=np.float64)
    key = hashlib.sha256(
        psf64.tobytes() + alpha64.tobytes() + str(m.shape).encode()).hexdigest()
    if key not in _cache:
        _cache[key] = _build(psf64, alpha64)
    nc = _cache[key]

    from concourse.bass_utils import run_bass_kernel_spmd
    w1 = _wpack(psf64)
    w2 = _wpack(psf64[::-1])
    r0 = _r0pack(psf64)
    in_maps = [{"m": np.ascontiguousarray(m[c * BC:(c + 1) * BC]).astype(np.float32),
                "w1": w1, "w2": w2, "r0": r0} for c in range(N_CORES)]
    res = run_bass_kernel_spmd(nc, in_maps, core_ids=list(range(N_CORES)))
    out = np.concatenate([res.results[c]["y"] for c in range(N_CORES)], axis=0)
    return out.astype(np.float32)



# revision 6
# speedup vs baseline: 1.1648x; 1.1648x over previous
"""Deep Richardson-Lucy deconvolution on 8 Trainium2 NeuronCores.

Strategy (per core, data-parallel batch shard of 512 rows):
- Transposed fp16 SBUF layout: l on partitions (tiles of 128), batch on
  the free dim (512).
- Offset tiling kills the halo matmuls: ratio tiles cover
  l in [128t-15, 128t+113) (offset -15).  Then conv1 for ratio tile t
  needs exactly s-tiles (t-1, t) and conv2 for s tile t needs exactly
  ratio tiles (t, t+1): each conv is 2 banded [128,128] matmuls, and
  adjacent output tiles pair into wide matmuls ([128,2,512] PSUM out,
  2 banks) so a conv pair is 2 matmuls covering 2 tiles.
- Engine split per RL iteration: PE convs; ACT reciprocal(+EPS) PSUM->
  ratio; GpSimd ratio *= m (SBUF only - GpSimd cannot touch PSUM);
  DVE update s = (psum2 + EPS) * s.
- In/out transposes ride the DMA xbar transpose engine (fp16).
"""
import hashlib
import numpy as np

EPS = 1e-6
P = 128
KTAPS = 31
PAD = 15
B_FULL, L = 4096, 8192
N_CORES = 8
BC = B_FULL // N_CORES          # 512 batch rows per core
NT = L // P                     # 64 s tiles
NR = NT + 1                     # 65 ratio tiles (offset -15 layout)
NITER = 10

_cache = {}


def _band(w31, shift):
    """W[p, c] = w31[p - c + shift] where index in [0, 31)."""
    W = np.zeros((P, P), dtype=np.float64)
    p = np.arange(P)[:, None]
    c = np.arange(P)[None, :]
    k = p - c + shift
    m = (k >= 0) & (k < KTAPS)
    W[m] = w31[k[m]]
    return W


def _wpack(psf64):
    """w1 = [W1a | W1b] for conv1 (psf), w2 = [W2a | W2b] for conv2
    (flipped psf).  conv1: ratio tile t (l' = 128t-15+c) from s tiles
    (t-1, t); conv2: s tile t (l' = 128t+c) from ratio tiles (t, t+1)."""
    wf = psf64[::-1]
    w1 = np.zeros((P, 2 * P), dtype=np.float16)
    w1[:, 0:P] = _band(psf64, -98)
    w1[:, P:2 * P] = _band(psf64, 30)
    w2 = np.zeros((P, 2 * P), dtype=np.float16)
    w2[:, 0:P] = _band(wf, 0)
    w2[:, P:2 * P] = _band(wf, 128)
    return w1, w2


def _r0pack(psf64):
    """Iteration-0 shortcut: s == 0.5 everywhere, so conv1(s) is a pure
    function of position.  Interior tiles: constant 0.5 (psf sums to 1).
    Edge tiles 0 and 64 (offset layout): per-partition values.
    Returns (r0_edge [P, 2] fp32, r0_const float)."""
    lp = np.arange(-PAD, L + PAD, dtype=np.int64)
    lo = np.maximum(lp - PAD, 0)
    hi = np.minimum(lp + PAD, L - 1)
    csum = np.concatenate([[0.0], np.cumsum(psf64)])
    # conv(0.5*ones)[l'] = 0.5 * sum_{k: 0<=l'+k-15<L} psf[k]
    c = 0.5 * (csum[hi - lp + PAD + 1] - csum[lo - lp + PAD])
    r = 1.0 / (c + EPS)
    r0e = np.zeros((P, 2), dtype=np.float32)
    r0e[:, 0] = r[0:P]                      # tile 0: l' = p - 15
    t64 = np.zeros(P, dtype=np.float64)
    n64 = min(L + PAD - (NT * P - PAD), P)  # valid entries of tile 64
    t64[:n64] = r[NT * P:NT * P + n64]
    r0e[:, 1] = t64                         # tile 64: l' = 8177 + p - 15+15
    return r0e, float(1.0 / (0.5 + EPS))


def _build(psf64, alpha64):
    import concourse.bass as bass
    import concourse.tile as tile
    from concourse import mybir
    import bass_rust

    F32 = mybir.dt.float32
    F16 = mybir.dt.float16

    class SafeTC(tile.TileContext):
        # this walrus build rejects >1 sync wait per CTRL-class instruction
        def _drain_and_barrier(self, tick_clock, wait_clock):
            gc = tick_clock.global_clock
            for i in range(len(gc)):
                if gc[i] > 0:
                    di = self.nc.sync.drain()
                    pc = bass_rust.VectorClock()
                    pc.require_at_least(i, gc[i])
                    wait_clock.add_sem_waits(di.ins, bass_rust.ScopedClock({None: pc}))
            self.nc.all_engine_barrier()
            popped = self.nc._tile_sem_poison_stack.pop()
            assert popped is self._sem_poison
            self.nc.clear_and_free_semaphores(list(self.sems.allocated().values()))
            self.nc.all_engine_barrier()

    def split_multi_waits(nc, max_waits=1):
        n_fixed = 0
        uid = [0]
        for f in nc.m.functions:
            for bb in f.blocks:
                out = []
                changed = False
                for inst in bb.instructions:
                    si = inst.sync_info
                    if si is not None:
                        sems = [w for w in si.on_wait
                                if str(getattr(w, "sync_type", "")) == "semaphore"]
                        other = [w for w in si.on_wait if w not in sems]
                        if len(sems) > max_waits:
                            keep = sems[-max_waits:]
                            for w in sems[:-max_waits]:
                                nop = mybir.InstNoOp(
                                    name=f"waitsplit_{uid[0]}", ins=[], outs=[])
                                uid[0] += 1
                                nop.engine = inst.engine
                                nop.sync_info = mybir.SyncInfo(
                                    on_wait=[w], on_update=[])
                                out.append(nop)
                            inst.sync_info = mybir.SyncInfo(
                                on_wait=other + keep,
                                on_update=list(si.on_update))
                            n_fixed += 1
                            changed = True
                    out.append(inst)
                if changed:
                    try:
                        bb.instructions = out
                    except Exception:
                        bb.instructions.clear()
                        bb.instructions.extend(out)
        return n_fixed

    def act_raw(nc, out, in_, func, bias=0.0, scale=1.0):
        eng = nc.scalar
        ins = [eng.lower_ap(in_),
               mybir.ImmediateValue(dtype=F32, value=float(bias)),
               mybir.ImmediateValue(dtype=F32, value=float(scale)),
               mybir.ImmediateValue(dtype=F32, value=0.0)]
        return eng.add_instruction(mybir.InstActivation(
            name=nc.get_next_instruction_name(), func=func, ins=ins,
            outs=[eng.lower_ap(out)]))

    alpha_is_one = bool(np.all(alpha64 == 1.0))
    r0e_np, r0c = _r0pack(psf64)

    nc = bass.Bass("TRN2", target_bir_lowering=False, debug=False,
                   num_devices=N_CORES)
    m_in = nc.dram_tensor("m", [BC, L], F32, kind="ExternalInput")
    w1_in = nc.dram_tensor("w1", [P, 2 * P], F16, kind="ExternalInput")
    w2_in = nc.dram_tensor("w2", [P, 2 * P], F16, kind="ExternalInput")
    r0_in = nc.dram_tensor("r0", [P, 2], F32, kind="ExternalInput")
    y_out = nc.dram_tensor("y", [BC, L], F32, kind="ExternalOutput")

    Rec = mybir.ActivationFunctionType.Reciprocal
    Ln = mybir.ActivationFunctionType.Ln
    Exp = mybir.ActivationFunctionType.Exp

    with SafeTC(nc) as tc:
        with tc.tile_pool(name="wpool", bufs=1) as wpool, \
             tc.tile_pool(name="mpool", bufs=1) as mpool, \
             tc.tile_pool(name="spool", bufs=1) as spool:
            w1 = wpool.tile([P, 2 * P], F16)
            nc.sync.dma_start(w1[:], w1_in[:])
            w2 = wpool.tile([P, 2 * P], F16)
            nc.sync.dma_start(w2[:], w2_in[:])
            r0e = wpool.tile([P, 2], F32)
            nc.sync.dma_start(r0e[:], r0_in[:])
            # mT: offset layout, tile t partition p <-> l = 128t - 15 + p
            mT = mpool.tile([P, NR, BC], F16)
            # s_buf: tile u holds s tile (u-1); u=0 and u=65 stay zero
            s_buf = spool.tile([P, NT + 2, BC], F16)
            nc.vector.memset(s_buf[:, 0, :], 0.0)
            nc.vector.memset(s_buf[:, NT + 1, :], 0.0)
            nc.vector.memset(s_buf[:, 1:NT + 1, :], 0.5)

            # ---- load m, cast fp16, DMA-xbar transpose into mT ----
            # staging covers l in [-15, 8305): 65 tiles of 128 = 8320 cols
            with tc.tile_pool(name="stage", bufs=1) as stage:
                for c in range(BC // P):
                    st32 = stage.tile([P, NR * P], F32, tag="st32")
                    nc.vector.memset(st32[:, 0:PAD], 0.0)
                    nc.vector.memset(st32[:, L + PAD:NR * P], 0.0)
                    nc.sync.dma_start(st32[:, PAD:L + PAD],
                                      m_in[c * P:(c + 1) * P, :])
                    st16 = stage.tile([P, NR * P], F16, tag="st16")
                    nc.vector.tensor_copy(st16[:], st32[:])
                    nc.sync.dma_start_transpose(
                        mT[:, 0:NR, c * P:(c + 1) * P], st16[:])

            # ---- RL iterations ----
            NG1 = 33            # conv1 groups: 32 pairs + single (tile 64)
            NG2 = 32            # conv2 groups: 32 pairs
            with tc.tile_pool(name="ratio", bufs=1) as rpool, \
                 tc.tile_pool(name="rt32p", bufs=1) as rtp, \
                 tc.tile_pool(name="aux", bufs=2) as auxp, \
                 tc.tile_pool(name="ps1p", bufs=2, space="PSUM") as pp1, \
                 tc.tile_pool(name="ps2p", bufs=2, space="PSUM") as pp2:
                ratio = rpool.tile([P, NR, BC], F16)
                rt32 = rtp.tile([P, 2, BC], F32)

                def c1_ratio(it, g):
                    if it == 0:
                        if g == 0:
                            nc.gpsimd.tensor_scalar(
                                out=ratio[:, 0, :], in0=mT[:, 0, :],
                                scalar1=r0e[:, 0:1], scalar2=None,
                                op0=mybir.AluOpType.mult)
                            nc.gpsimd.tensor_scalar(
                                out=ratio[:, 1, :], in0=mT[:, 1, :],
                                scalar1=r0c, scalar2=None,
                                op0=mybir.AluOpType.mult)
                        elif g == NG1 - 1:
                            nc.gpsimd.tensor_scalar(
                                out=ratio[:, NR - 1, :], in0=mT[:, NR - 1, :],
                                scalar1=r0e[:, 1:2], scalar2=None,
                                op0=mybir.AluOpType.mult)
                        else:
                            t = 2 * g
                            nc.gpsimd.tensor_scalar(
                                out=ratio[:, t:t + 2, :], in0=mT[:, t:t + 2, :],
                                scalar1=r0c, scalar2=None,
                                op0=mybir.AluOpType.mult)
                        return
                    if g == NG1 - 1:
                        # single ratio tile 64 (k-tile1 is zero pad - skip)
                        ps = pp1.tile([P, 2, BC], F32, tag="ps1")
                        nc.tensor.matmul(ps[:, 0, :], w1[:, 0:P],
                                         s_buf[:, NT, :], start=True, stop=True)
                        act_raw(nc, rt32[:, 0, :], ps[:, 0, :], Rec, bias=EPS)
                        nc.gpsimd.tensor_mul(ratio[:, NR - 1, :],
                                             mT[:, NR - 1, :], rt32[:, 0, :])
                        return
                    t = 2 * g
                    ps = pp1.tile([P, 2, BC], F32, tag="ps1")
                    # matmul out must fit one PSUM bank: 2 narrow MMs per
                    # weight, same-weight back-to-back
                    nc.tensor.matmul(ps[:, 0, :], w1[:, 0:P], s_buf[:, t, :],
                                     start=True, stop=False,
                                     skip_group_check=True)
                    nc.tensor.matmul(ps[:, 1, :], w1[:, 0:P],
                                     s_buf[:, t + 1, :],
                                     start=True, stop=False,
                                     skip_group_check=True)
                    nc.tensor.matmul(ps[:, 0, :], w1[:, P:2 * P],
                                     s_buf[:, t + 1, :],
                                     start=False, stop=True,
                                     skip_group_check=True)
                    nc.tensor.matmul(ps[:, 1, :], w1[:, P:2 * P],
                                     s_buf[:, t + 2, :],
                                     start=False, stop=True,
                                     skip_group_check=True)
                    if g == 0:
                        # edge tile: recip can exceed fp16 range where m==0
                        act_raw(nc, rt32[:], ps[:], Rec, bias=EPS)
                        nc.gpsimd.tensor_mul(ratio[:, 0:2, :],
                                             mT[:, 0:2, :], rt32[:])
                    else:
                        act_raw(nc, ratio[:, t:t + 2, :], ps[:], Rec, bias=EPS)
                        nc.gpsimd.tensor_mul(ratio[:, t:t + 2, :],
                                             ratio[:, t:t + 2, :],
                                             mT[:, t:t + 2, :])

                def c2_update(it, j):
                    t = 2 * j
                    ps = pp2.tile([P, 2, BC], F32, tag="ps2")
                    nc.tensor.matmul(ps[:, 0, :], w2[:, 0:P], ratio[:, t, :],
                                     start=True, stop=False,
                                     skip_group_check=True)
                    nc.tensor.matmul(ps[:, 1, :], w2[:, 0:P],
                                     ratio[:, t + 1, :],
                                     start=True, stop=False,
                                     skip_group_check=True)
                    nc.tensor.matmul(ps[:, 0, :], w2[:, P:2 * P],
                                     ratio[:, t + 1, :],
                                     start=False, stop=True,
                                     skip_group_check=True)
                    nc.tensor.matmul(ps[:, 1, :], w2[:, P:2 * P],
                                     ratio[:, t + 2, :],
                                     start=False, stop=True,
                                     skip_group_check=True)
                    dst = s_buf[:, t + 1:t + 3, :]
                    if alpha_is_one:
                        nc.vector.scalar_tensor_tensor(
                            out=dst, in0=ps[:], scalar=EPS, in1=dst,
                            op0=mybir.AluOpType.add,
                            op1=mybir.AluOpType.mult)
                    else:
                        lg = auxp.tile([P, 2, BC], F32, tag="lg")
                        act_raw(nc, lg[:], ps[:], Ln, bias=EPS)
                        cp = auxp.tile([P, 2, BC], F16, tag="cp")
                        act_raw(nc, cp[:], lg[:], Exp,
                                scale=float(alpha64[it]))
                        nc.vector.tensor_mul(dst, dst, cp[:])

                for it in range(NITER):
                    c1_ratio(it, 0)
                    c1_ratio(it, 1)
                    for j in range(NG2):
                        if j + 2 < NG1:
                            c1_ratio(it, j + 2)
                        c2_update(it, j)

            # ---- transpose back + cast fp32 + store ----
            with tc.tile_pool(name="outp", bufs=1) as outp:
                for q in range(4):
                    sn16 = outp.tile([P, NT, P], F16, tag="sn16")
                    nc.sync.dma_start_transpose(
                        sn16[:], s_buf[:, 1 + q * 16:1 + (q + 1) * 16, :])
                    sn32 = outp.tile([P, NT, P], F32, tag="sn32")
                    nc.vector.tensor_copy(sn32[:], sn16[:])
                    sn32r = sn32.rearrange("p (tl bc) lp -> p tl bc lp",
                                           tl=16, bc=4)
                    for bc in range(4):
                        ydst = y_out[bc * P:(bc + 1) * P,
                                     q * 2048:(q + 1) * 2048].rearrange(
                            "p (tl lp) -> p tl lp", lp=P)
                        nc.sync.dma_start(ydst, sn32r[:, :, bc, :])

    split_multi_waits(nc)
    return nc


def kernel(m, psf, alpha):
    m = np.asarray(m)
    psf64 = np.asarray(psf, dtype=np.float64)
    alpha64 = np.asarray(alpha, dtype=np.float64)
    key = hashlib.sha256(
        psf64.tobytes() + alpha64.tobytes() + str(m.shape).encode()).hexdigest()
    if key not in _cache:
        _cache[key] = _build(psf64, alpha64)
    nc = _cache[key]

    from concourse.bass_utils import run_bass_kernel_spmd
    w1, w2 = _wpack(psf64)
    r0e, _ = _r0pack(psf64)
    in_maps = [{"m": np.ascontiguousarray(m[c * BC:(c + 1) * BC]).astype(np.float32),
                "w1": w1, "w2": w2, "r0": r0e} for c in range(N_CORES)]
    res = run_bass_kernel_spmd(nc, in_maps, core_ids=list(range(N_CORES)))
    out = np.concatenate([res.results[c]["y"] for c in range(N_CORES)], axis=0)
    return out.astype(np.float32)


# revision 8
# speedup vs baseline: 1.7153x; 1.4726x over previous
"""Deep Richardson-Lucy deconvolution on 8 Trainium2 NeuronCores.

Strategy (per core, data-parallel batch shard of 512 rows):
- Transposed fp16 SBUF layout: l on partitions (tiles of 128), batch on
  the free dim (512).
- Offset tiling kills the halo matmuls: ratio tiles cover
  l in [128t-15, 128t+113) (offset -15).  Then conv1 for ratio tile t
  needs exactly s-tiles (t-1, t) and conv2 for s tile t needs exactly
  ratio tiles (t, t+1): each conv is 2 banded [128,128] matmuls, and
  adjacent output tiles pair into wide matmuls ([128,2,512] PSUM out,
  2 banks) so a conv pair is 2 matmuls covering 2 tiles.
- Engine split per RL iteration: PE convs; ACT reciprocal(+EPS) PSUM->
  ratio; GpSimd ratio *= m (SBUF only - GpSimd cannot touch PSUM);
  DVE update s = (psum2 + EPS) * s.
- In/out transposes ride the DMA xbar transpose engine (fp16).
"""
import hashlib
import numpy as np

EPS = 1e-6
P = 128
KTAPS = 31
PAD = 15
B_FULL, L = 4096, 8192
N_CORES = 8
BC = B_FULL // N_CORES          # 512 batch rows per core
NT = L // P                     # 64 s tiles
NR = NT + 1                     # 65 ratio tiles (offset -15 layout)
NITER = 10

_cache = {}


def _band(w31, shift):
    """W[p, c] = w31[p - c + shift] where index in [0, 31)."""
    W = np.zeros((P, P), dtype=np.float64)
    p = np.arange(P)[:, None]
    c = np.arange(P)[None, :]
    k = p - c + shift
    m = (k >= 0) & (k < KTAPS)
    W[m] = w31[k[m]]
    return W


def _wpack(psf64):
    """w1 = [W1a | W1b] for conv1 (psf), w2 = [W2a | W2b] for conv2
    (flipped psf).  conv1: ratio tile t (l' = 128t-15+c) from s tiles
    (t-1, t); conv2: s tile t (l' = 128t+c) from ratio tiles (t, t+1)."""
    wf = psf64[::-1]
    w1 = np.zeros((P, 2 * P), dtype=np.float16)
    w1[:, 0:P] = _band(psf64, -98)
    w1[:, P:2 * P] = _band(psf64, 30)
    w2 = np.zeros((P, 2 * P), dtype=np.float16)
    w2[:, 0:P] = _band(wf, 0)
    w2[:, P:2 * P] = _band(wf, 128)
    return w1, w2


def _r0pack(psf64):
    """Iteration-0 shortcut: s == 0.5 everywhere, so conv1(s) is a pure
    function of position.  Interior tiles: constant 0.5 (psf sums to 1).
    Edge tiles 0 and 64 (offset layout): per-partition values.
    Returns (r0_edge [P, 2] fp32, r0_const float)."""
    lp = np.arange(-PAD, L + PAD, dtype=np.int64)
    lo = np.maximum(lp - PAD, 0)
    hi = np.minimum(lp + PAD, L - 1)
    csum = np.concatenate([[0.0], np.cumsum(psf64)])
    # conv(0.5*ones)[l'] = 0.5 * sum_{k: 0<=l'+k-15<L} psf[k]
    c = 0.5 * (csum[hi - lp + PAD + 1] - csum[lo - lp + PAD])
    r = 1.0 / (c + EPS)
    r0e = np.zeros((P, 2), dtype=np.float32)
    r0e[:, 0] = r[0:P]                      # tile 0: l' = p - 15
    t64 = np.zeros(P, dtype=np.float64)
    n64 = min(L + PAD - (NT * P - PAD), P)  # valid entries of tile 64
    t64[:n64] = r[NT * P:NT * P + n64]
    r0e[:, 1] = t64                         # tile 64: l' = 8177 + p - 15+15
    return r0e, float(1.0 / (0.5 + EPS))


def _build(psf64, alpha64):
    import concourse.bass as bass
    import concourse.tile as tile
    from concourse import mybir
    import bass_rust

    F32 = mybir.dt.float32
    F16 = mybir.dt.float16

    class SafeTC(tile.TileContext):
        # this walrus build rejects >1 sync wait per CTRL-class instruction
        def _drain_and_barrier(self, tick_clock, wait_clock):
            gc = tick_clock.global_clock
            for i in range(len(gc)):
                if gc[i] > 0:
                    di = self.nc.sync.drain()
                    pc = bass_rust.VectorClock()
                    pc.require_at_least(i, gc[i])
                    wait_clock.add_sem_waits(di.ins, bass_rust.ScopedClock({None: pc}))
            self.nc.all_engine_barrier()
            popped = self.nc._tile_sem_poison_stack.pop()
            assert popped is self._sem_poison
            self.nc.clear_and_free_semaphores(list(self.sems.allocated().values()))
            self.nc.all_engine_barrier()

    def split_multi_waits(nc, max_waits=1):
        n_fixed = 0
        uid = [0]
        for f in nc.m.functions:
            for bb in f.blocks:
                out = []
                changed = False
                for inst in bb.instructions:
                    si = inst.sync_info
                    if si is not None:
                        sems = [w for w in si.on_wait
                                if str(getattr(w, "sync_type", "")) == "semaphore"]
                        other = [w for w in si.on_wait if w not in sems]
                        if len(sems) > max_waits:
                            keep = sems[-max_waits:]
                            for w in sems[:-max_waits]:
                                nop = mybir.InstNoOp(
                                    name=f"waitsplit_{uid[0]}", ins=[], outs=[])
                                uid[0] += 1
                                nop.engine = inst.engine
                                nop.sync_info = mybir.SyncInfo(
                                    on_wait=[w], on_update=[])
                                out.append(nop)
                            inst.sync_info = mybir.SyncInfo(
                                on_wait=other + keep,
                                on_update=list(si.on_update))
                            n_fixed += 1
                            changed = True
                    out.append(inst)
                if changed:
                    try:
                        bb.instructions = out
                    except Exception:
                        bb.instructions.clear()
                        bb.instructions.extend(out)
        return n_fixed

    def act_raw(nc, out, in_, func, bias=0.0, scale=1.0):
        eng = nc.scalar
        ins = [eng.lower_ap(in_),
               mybir.ImmediateValue(dtype=F32, value=float(bias)),
               mybir.ImmediateValue(dtype=F32, value=float(scale)),
               mybir.ImmediateValue(dtype=F32, value=0.0)]
        return eng.add_instruction(mybir.InstActivation(
            name=nc.get_next_instruction_name(), func=func, ins=ins,
            outs=[eng.lower_ap(out)]))

    alpha_is_one = bool(np.all(alpha64 == 1.0))
    r0e_np, r0c = _r0pack(psf64)

    nc = bass.Bass("TRN2", target_bir_lowering=False, debug=False,
                   num_devices=N_CORES)
    m_in = nc.dram_tensor("m", [BC, L], F32, kind="ExternalInput")
    w1_in = nc.dram_tensor("w1", [P, 2 * P], F16, kind="ExternalInput")
    w2_in = nc.dram_tensor("w2", [P, 2 * P], F16, kind="ExternalInput")
    r0_in = nc.dram_tensor("r0", [P, 2], F32, kind="ExternalInput")
    y_out = nc.dram_tensor("y", [BC, L], F32, kind="ExternalOutput")

    Rec = mybir.ActivationFunctionType.Reciprocal
    Ln = mybir.ActivationFunctionType.Ln
    Exp = mybir.ActivationFunctionType.Exp

    with SafeTC(nc) as tc:
        with tc.tile_pool(name="wpool", bufs=1) as wpool, \
             tc.tile_pool(name="mpool", bufs=1) as mpool, \
             tc.tile_pool(name="spool", bufs=1) as spool:
            w1 = wpool.tile([P, 2 * P], F16)
            nc.sync.dma_start(w1[:], w1_in[:])
            w2 = wpool.tile([P, 2 * P], F16)
            nc.sync.dma_start(w2[:], w2_in[:])
            r0e = wpool.tile([P, 2], F32)
            nc.sync.dma_start(r0e[:], r0_in[:])
            # mT: offset layout, tile t partition p <-> l = 128t - 15 + p
            mT = mpool.tile([P, NR, BC], F16)
            # s_buf: tile u holds s tile (u-1); u=0 and u=65 stay zero
            s_buf = spool.tile([P, NT + 2, BC], F16)
            nc.vector.memset(s_buf[:, 0, :], 0.0)
            nc.vector.memset(s_buf[:, NT + 1, :], 0.0)
            nc.vector.memset(s_buf[:, 1:NT + 1, :], 0.5)

            # ---- load m, cast fp16, DMA-xbar transpose into mT ----
            # staging covers l in [-15, 8305): 65 tiles of 128 = 8320 cols
            with tc.tile_pool(name="stage", bufs=1) as stage:
                for c in range(BC // P):
                    st32 = stage.tile([P, NR * P], F32, tag="st32")
                    nc.vector.memset(st32[:, 0:PAD], 0.0)
                    nc.vector.memset(st32[:, L + PAD:NR * P], 0.0)
                    nc.sync.dma_start(st32[:, PAD:L + PAD],
                                      m_in[c * P:(c + 1) * P, :])
                    st16 = stage.tile([P, NR * P], F16, tag="st16")
                    nc.vector.tensor_copy(st16[:], st32[:])
                    nc.sync.dma_start_transpose(
                        mT[:, 0:NR, c * P:(c + 1) * P], st16[:])

            # ---- RL iterations ----
            NG1 = 33            # conv1 groups: 32 pairs + single (tile 64)
            NG2 = 32            # conv2 groups: 32 pairs
            with tc.tile_pool(name="ratio", bufs=1) as rpool, \
                 tc.tile_pool(name="rt32p", bufs=1) as rtp, \
                 tc.tile_pool(name="aux", bufs=2) as auxp, \
                 tc.tile_pool(name="ps1p", bufs=2, space="PSUM") as pp1, \
                 tc.tile_pool(name="ps2p", bufs=2, space="PSUM") as pp2:
                ratio = rpool.tile([P, NR, BC], F16)
                rt32 = rtp.tile([P, 2, BC], F32)

                def c1_ratio(it, g):
                    if it == 0:
                        if g == 0:
                            nc.vector.tensor_scalar(
                                out=ratio[:, 0, :], in0=mT[:, 0, :],
                                scalar1=r0e[:, 0:1], scalar2=None,
                                op0=mybir.AluOpType.mult)
                            nc.vector.tensor_scalar(
                                out=ratio[:, 1, :], in0=mT[:, 1, :],
                                scalar1=r0c, scalar2=None,
                                op0=mybir.AluOpType.mult)
                        elif g == NG1 - 1:
                            nc.vector.tensor_scalar(
                                out=ratio[:, NR - 1, :], in0=mT[:, NR - 1, :],
                                scalar1=r0e[:, 1:2], scalar2=None,
                                op0=mybir.AluOpType.mult)
                        else:
                            t = 2 * g
                            nc.vector.tensor_scalar(
                                out=ratio[:, t:t + 2, :], in0=mT[:, t:t + 2, :],
                                scalar1=r0c, scalar2=None,
                                op0=mybir.AluOpType.mult)
                        return
                    if g == NG1 - 1:
                        # single ratio tile 64 (k-tile1 is zero pad - skip)
                        ps = pp1.tile([P, 2, BC], F32, tag="ps1")
                        nc.tensor.matmul(ps[:, 0, :], w1[:, 0:P],
                                         s_buf[:, NT, :], start=True, stop=True)
                        act_raw(nc, rt32[:, 0, :], ps[:, 0, :], Rec, bias=EPS)
                        nc.vector.tensor_mul(ratio[:, NR - 1, :],
                                             mT[:, NR - 1, :], rt32[:, 0, :])
                        return
                    t = 2 * g
                    ps = pp1.tile([P, 2, BC], F32, tag="ps1")
                    # matmul out must fit one PSUM bank: 2 narrow MMs per
                    # weight, same-weight back-to-back
                    nc.tensor.matmul(ps[:, 0, :], w1[:, 0:P], s_buf[:, t, :],
                                     start=True, stop=False,
                                     skip_group_check=True)
                    nc.tensor.matmul(ps[:, 1, :], w1[:, 0:P],
                                     s_buf[:, t + 1, :],
                                     start=True, stop=False,
                                     skip_group_check=True)
                    nc.tensor.matmul(ps[:, 0, :], w1[:, P:2 * P],
                                     s_buf[:, t + 1, :],
                                     start=False, stop=True,
                                     skip_group_check=True)
                    nc.tensor.matmul(ps[:, 1, :], w1[:, P:2 * P],
                                     s_buf[:, t + 2, :],
                                     start=False, stop=True,
                                     skip_group_check=True)
                    if g == 0:
                        # edge tile: recip can exceed fp16 range where m==0
                        act_raw(nc, rt32[:], ps[:], Rec, bias=EPS)
                        nc.vector.tensor_mul(ratio[:, 0:2, :],
                                             mT[:, 0:2, :], rt32[:])
                    elif g % 2 == 0:
                        act_raw(nc, ratio[:, t:t + 2, :], ps[:], Rec, bias=EPS)
                        nc.gpsimd.tensor_mul(ratio[:, t:t + 2, :],
                                             ratio[:, t:t + 2, :],
                                             mT[:, t:t + 2, :])
                    else:
                        # DVE, split narrow: tile t first (conv2 group g-1
                        # waits on it), then t+1
                        act_raw(nc, ratio[:, t:t + 2, :], ps[:], Rec, bias=EPS)
                        nc.vector.tensor_mul(ratio[:, t, :], ratio[:, t, :],
                                             mT[:, t, :])
                        nc.vector.tensor_mul(ratio[:, t + 1, :],
                                             ratio[:, t + 1, :],
                                             mT[:, t + 1, :])

                def c2_update(it, j):
                    t = 2 * j
                    ps = pp2.tile([P, 2, BC], F32, tag="ps2")
                    nc.tensor.matmul(ps[:, 0, :], w2[:, 0:P], ratio[:, t, :],
                                     start=True, stop=False,
                                     skip_group_check=True)
                    nc.tensor.matmul(ps[:, 1, :], w2[:, 0:P],
                                     ratio[:, t + 1, :],
                                     start=True, stop=False,
                                     skip_group_check=True)
                    nc.tensor.matmul(ps[:, 0, :], w2[:, P:2 * P],
                                     ratio[:, t + 1, :],
                                     start=False, stop=True,
                                     skip_group_check=True)
                    nc.tensor.matmul(ps[:, 1, :], w2[:, P:2 * P],
                                     ratio[:, t + 2, :],
                                     start=False, stop=True,
                                     skip_group_check=True)
                    dst = s_buf[:, t + 1:t + 3, :]
                    if alpha_is_one:
                        nc.vector.scalar_tensor_tensor(
                            out=dst, in0=ps[:], scalar=EPS, in1=dst,
                            op0=mybir.AluOpType.add,
                            op1=mybir.AluOpType.mult)
                    else:
                        lg = auxp.tile([P, 2, BC], F32, tag="lg")
                        act_raw(nc, lg[:], ps[:], Ln, bias=EPS)
                        cp = auxp.tile([P, 2, BC], F16, tag="cp")
                        act_raw(nc, cp[:], lg[:], Exp,
                                scale=float(alpha64[it]))
                        nc.vector.tensor_mul(dst, dst, cp[:])

                for it in range(NITER):
                    c1_ratio(it, 0)
                    c1_ratio(it, 1)
                    for j in range(NG2):
                        if j + 2 < NG1:
                            c1_ratio(it, j + 2)
                        c2_update(it, j)

            # ---- transpose back + cast fp32 + store ----
            with tc.tile_pool(name="outp", bufs=1) as outp:
                for q in range(4):
                    sn16 = outp.tile([P, NT, P], F16, tag="sn16")
                    nc.sync.dma_start_transpose(
                        sn16[:], s_buf[:, 1 + q * 16:1 + (q + 1) * 16, :])
                    sn32 = outp.tile([P, NT, P], F32, tag="sn32")
                    nc.vector.tensor_copy(sn32[:], sn16[:])
                    sn32r = sn32.rearrange("p (tl bc) lp -> p tl bc lp",
                                           tl=16, bc=4)
                    for bc in range(4):
                        ydst = y_out[bc * P:(bc + 1) * P,
                                     q * 2048:(q + 1) * 2048].rearrange(
                            "p (tl lp) -> p tl lp", lp=P)
                        nc.sync.dma_start(ydst, sn32r[:, :, bc, :])

    split_multi_waits(nc)
    return nc


def kernel(m, psf, alpha):
    m = np.asarray(m)
    psf64 = np.asarray(psf, dtype=np.float64)
    alpha64 = np.asarray(alpha, dtype=np.float64)
    key = hashlib.sha256(
        psf64.tobytes() + alpha64.tobytes() + str(m.shape).encode()).hexdigest()
    if key not in _cache:
        _cache[key] = _build(psf64, alpha64)
    nc = _cache[key]

    from concourse.bass_utils import run_bass_kernel_spmd
    w1, w2 = _wpack(psf64)
    r0e, _ = _r0pack(psf64)
    in_maps = [{"m": np.ascontiguousarray(m[c * BC:(c + 1) * BC]).astype(np.float32),
                "w1": w1, "w2": w2, "r0": r0e} for c in range(N_CORES)]
    res = run_bass_kernel_spmd(nc, in_maps, core_ids=list(range(N_CORES)))
    out = np.concatenate([res.results[c]["y"] for c in range(N_CORES)], axis=0)
    return out.astype(np.float32)


# revision 10
# speedup vs baseline: 2.0063x; 1.1697x over previous
"""Deep Richardson-Lucy deconvolution on 8 Trainium2 NeuronCores.

Strategy (per core, data-parallel batch shard of 512 rows):
- Transposed fp16 SBUF layout: l on partitions (tiles of 128), batch on
  the free dim (512).
- Offset tiling kills the halo matmuls: ratio tiles cover
  l in [128t-15, 128t+113) (offset -15).  Then conv1 for ratio tile t
  needs exactly s-tiles (t-1, t) and conv2 for s tile t needs exactly
  ratio tiles (t, t+1): each conv is 2 banded [128,128] matmuls, and
  adjacent output tiles pair into wide matmuls ([128,2,512] PSUM out,
  2 banks) so a conv pair is 2 matmuls covering 2 tiles.
- Engine split per RL iteration: PE convs; ACT reciprocal(+EPS) PSUM->
  ratio; GpSimd ratio *= m (SBUF only - GpSimd cannot touch PSUM);
  DVE update s = (psum2 + EPS) * s.
- In/out transposes ride the DMA xbar transpose engine (fp16).
"""
import hashlib
import numpy as np

EPS = 1e-6
P = 128
KTAPS = 31
PAD = 15
B_FULL, L = 4096, 8192
N_CORES = 8
BC = B_FULL // N_CORES          # 512 batch rows per core
NT = L // P                     # 64 s tiles
NR = NT + 1                     # 65 ratio tiles (offset -15 layout)
NITER = 10

_cache = {}


def _band(w31, shift):
    """W[p, c] = w31[p - c + shift] where index in [0, 31)."""
    W = np.zeros((P, P), dtype=np.float64)
    p = np.arange(P)[:, None]
    c = np.arange(P)[None, :]
    k = p - c + shift
    m = (k >= 0) & (k < KTAPS)
    W[m] = w31[k[m]]
    return W


def _wpack(psf64):
    """w1 = [W1a | W1b] for conv1 (psf), w2 = [W2a | W2b] for conv2
    (flipped psf).  conv1: ratio tile t (l' = 128t-15+c) from s tiles
    (t-1, t); conv2: s tile t (l' = 128t+c) from ratio tiles (t, t+1)."""
    wf = psf64[::-1]
    w1 = np.zeros((P, 2 * P), dtype=np.float16)
    w1[:, 0:P] = _band(psf64, -98)
    w1[:, P:2 * P] = _band(psf64, 30)
    w2 = np.zeros((P, 2 * P), dtype=np.float16)
    w2[:, 0:P] = _band(wf, 0)
    w2[:, P:2 * P] = _band(wf, 128)
    return w1, w2


def _r0pack(psf64):
    """Iteration-0 shortcut: s == 0.5 everywhere, so conv1(s) is a pure
    function of position.  Interior tiles: constant 0.5 (psf sums to 1).
    Edge tiles 0 and 64 (offset layout): per-partition values.
    Returns (r0_edge [P, 2] fp32, r0_const float)."""
    lp = np.arange(-PAD, L + PAD, dtype=np.int64)
    lo = np.maximum(lp - PAD, 0)
    hi = np.minimum(lp + PAD, L - 1)
    csum = np.concatenate([[0.0], np.cumsum(psf64)])
    # conv(0.5*ones)[l'] = 0.5 * sum_{k: 0<=l'+k-15<L} psf[k]
    c = 0.5 * (csum[hi - lp + PAD + 1] - csum[lo - lp + PAD])
    r = 1.0 / (c + EPS)
    r0e = np.zeros((P, 2), dtype=np.float32)
    r0e[:, 0] = r[0:P]                      # tile 0: l' = p - 15
    t64 = np.zeros(P, dtype=np.float64)
    n64 = min(L + PAD - (NT * P - PAD), P)  # valid entries of tile 64
    t64[:n64] = r[NT * P:NT * P + n64]
    r0e[:, 1] = t64                         # tile 64: l' = 8177 + p - 15+15
    return r0e, float(1.0 / (0.5 + EPS))


def _build(psf64, alpha64):
    import concourse.bass as bass
    import concourse.tile as tile
    from concourse import mybir
    import bass_rust

    F32 = mybir.dt.float32
    F16 = mybir.dt.float16

    class SafeTC(tile.TileContext):
        # this walrus build rejects >1 sync wait per CTRL-class instruction
        def _drain_and_barrier(self, tick_clock, wait_clock):
            gc = tick_clock.global_clock
            for i in range(len(gc)):
                if gc[i] > 0:
                    di = self.nc.sync.drain()
                    pc = bass_rust.VectorClock()
                    pc.require_at_least(i, gc[i])
                    wait_clock.add_sem_waits(di.ins, bass_rust.ScopedClock({None: pc}))
            self.nc.all_engine_barrier()
            popped = self.nc._tile_sem_poison_stack.pop()
            assert popped is self._sem_poison
            self.nc.clear_and_free_semaphores(list(self.sems.allocated().values()))
            self.nc.all_engine_barrier()

    def split_multi_waits(nc, max_waits=1):
        n_fixed = 0
        uid = [0]
        for f in nc.m.functions:
            for bb in f.blocks:
                out = []
                changed = False
                for inst in bb.instructions:
                    si = inst.sync_info
                    if si is not None:
                        sems = [w for w in si.on_wait
                                if str(getattr(w, "sync_type", "")) == "semaphore"]
                        other = [w for w in si.on_wait if w not in sems]
                        if len(sems) > max_waits:
                            keep = sems[-max_waits:]
                            for w in sems[:-max_waits]:
                                nop = mybir.InstNoOp(
                                    name=f"waitsplit_{uid[0]}", ins=[], outs=[])
                                uid[0] += 1
                                nop.engine = inst.engine
                                nop.sync_info = mybir.SyncInfo(
                                    on_wait=[w], on_update=[])
                                out.append(nop)
                            inst.sync_info = mybir.SyncInfo(
                                on_wait=other + keep,
                                on_update=list(si.on_update))
                            n_fixed += 1
                            changed = True
                    out.append(inst)
                if changed:
                    try:
                        bb.instructions = out
                    except Exception:
                        bb.instructions.clear()
                        bb.instructions.extend(out)
        return n_fixed

    def act_raw(nc, out, in_, func, bias=0.0, scale=1.0):
        eng = nc.scalar
        ins = [eng.lower_ap(in_),
               mybir.ImmediateValue(dtype=F32, value=float(bias)),
               mybir.ImmediateValue(dtype=F32, value=float(scale)),
               mybir.ImmediateValue(dtype=F32, value=0.0)]
        return eng.add_instruction(mybir.InstActivation(
            name=nc.get_next_instruction_name(), func=func, ins=ins,
            outs=[eng.lower_ap(out)]))

    alpha_is_one = bool(np.all(alpha64 == 1.0))
    r0e_np, r0c = _r0pack(psf64)

    nc = bass.Bass("TRN2", target_bir_lowering=False, debug=False,
                   num_devices=N_CORES)
    m_in = nc.dram_tensor("m", [BC, L], F32, kind="ExternalInput")
    w1_in = nc.dram_tensor("w1", [P, 2 * P], F16, kind="ExternalInput")
    w2_in = nc.dram_tensor("w2", [P, 2 * P], F16, kind="ExternalInput")
    r0_in = nc.dram_tensor("r0", [P, 2], F32, kind="ExternalInput")
    y_out = nc.dram_tensor("y", [BC, L], F32, kind="ExternalOutput")

    Rec = mybir.ActivationFunctionType.Reciprocal
    Ln = mybir.ActivationFunctionType.Ln
    Exp = mybir.ActivationFunctionType.Exp

    with SafeTC(nc) as tc:
        with tc.tile_pool(name="wpool", bufs=1) as wpool, \
             tc.tile_pool(name="mpool", bufs=1) as mpool, \
             tc.tile_pool(name="spool", bufs=1) as spool:
            w1 = wpool.tile([P, 2 * P], F16)
            nc.sync.dma_start(w1[:], w1_in[:])
            w2 = wpool.tile([P, 2 * P], F16)
            nc.sync.dma_start(w2[:], w2_in[:])
            r0e = wpool.tile([P, 2], F32)
            nc.sync.dma_start(r0e[:], r0_in[:])
            # mT: offset layout, tile t partition p <-> l = 128t - 15 + p
            mT = mpool.tile([P, NR, BC], F16)
            # s_buf: tile u holds s tile (u-1); u=0 and u=65 stay zero
            s_buf = spool.tile([P, NT + 2, BC], F16)
            nc.gpsimd.memset(s_buf[:, 0, :], 0.0)
            nc.gpsimd.memset(s_buf[:, NT + 1, :], 0.0)
            nc.gpsimd.memset(s_buf[:, 1:NT + 1, :], 0.5)

            # ---- load m, cast fp16, DMA-xbar transpose into mT ----
            # staging covers l in [-15, 8305): 65 tiles of 128 = 8320 cols
            # st16 double-buffered so cast(c+1) overlaps transpose(c);
            # casts alternate ACT/DVE
            with tc.tile_pool(name="stage32", bufs=1) as stage32, \
                 tc.tile_pool(name="stage16", bufs=2) as stage16:
                for c in range(BC // P):
                    st32 = stage32.tile([P, NR * P], F32, tag="st32")
                    nc.gpsimd.memset(st32[:, 0:PAD], 0.0)
                    nc.gpsimd.memset(st32[:, L + PAD:NR * P], 0.0)
                    nc.sync.dma_start(st32[:, PAD:L + PAD],
                                      m_in[c * P:(c + 1) * P, :])
                    st16 = stage16.tile([P, NR * P], F16, tag="st16")
                    if c % 2 == 0:
                        act_raw(nc, st16[:], st32[:],
                                mybir.ActivationFunctionType.Copy)
                    else:
                        nc.vector.tensor_copy(st16[:], st32[:])
                    nc.sync.dma_start_transpose(
                        mT[:, 0:NR, c * P:(c + 1) * P], st16[:])

            # ---- RL iterations ----
            NG1 = 33            # conv1 groups: 32 pairs + single (tile 64)
            NG2 = 32            # conv2 groups: 32 pairs
            with tc.tile_pool(name="ratio", bufs=1) as rpool, \
                 tc.tile_pool(name="rt32p", bufs=1) as rtp, \
                 tc.tile_pool(name="aux", bufs=2) as auxp, \
                 tc.tile_pool(name="ps1p", bufs=2, space="PSUM") as pp1, \
                 tc.tile_pool(name="ps2p", bufs=2, space="PSUM") as pp2:
                ratio = rpool.tile([P, NR, BC], F16)
                rt32 = rtp.tile([P, 2, BC], F32)

                def c1_ratio(it, g):
                    if it == 0:
                        if g == 0:
                            nc.vector.tensor_scalar(
                                out=ratio[:, 0, :], in0=mT[:, 0, :],
                                scalar1=r0e[:, 0:1], scalar2=None,
                                op0=mybir.AluOpType.mult)
                            nc.vector.tensor_scalar(
                                out=ratio[:, 1, :], in0=mT[:, 1, :],
                                scalar1=r0c, scalar2=None,
                                op0=mybir.AluOpType.mult)
                        elif g == NG1 - 1:
                            nc.vector.tensor_scalar(
                                out=ratio[:, NR - 1, :], in0=mT[:, NR - 1, :],
                                scalar1=r0e[:, 1:2], scalar2=None,
                                op0=mybir.AluOpType.mult)
                        else:
                            t = 2 * g
                            nc.vector.tensor_scalar(
                                out=ratio[:, t:t + 2, :], in0=mT[:, t:t + 2, :],
                                scalar1=r0c, scalar2=None,
                                op0=mybir.AluOpType.mult)
                        return
                    if g == NG1 - 1:
                        # single ratio tile 64 (k-tile1 is zero pad - skip)
                        ps = pp1.tile([P, 2, BC], F32, tag="ps1")
                        nc.tensor.matmul(ps[:, 0, :], w1[:, 0:P],
                                         s_buf[:, NT, :], start=True, stop=True)
                        act_raw(nc, rt32[:, 0, :], ps[:, 0, :], Rec, bias=EPS)
                        nc.vector.tensor_mul(ratio[:, NR - 1, :],
                                             mT[:, NR - 1, :], rt32[:, 0, :])
                        return
                    t = 2 * g
                    ps = pp1.tile([P, 2, BC], F32, tag="ps1")
                    # matmul out must fit one PSUM bank: 2 narrow MMs per
                    # weight, same-weight back-to-back
                    nc.tensor.matmul(ps[:, 0, :], w1[:, 0:P], s_buf[:, t, :],
                                     start=True, stop=False,
                                     skip_group_check=True)
                    nc.tensor.matmul(ps[:, 1, :], w1[:, 0:P],
                                     s_buf[:, t + 1, :],
                                     start=True, stop=False,
                                     skip_group_check=True)
                    nc.tensor.matmul(ps[:, 0, :], w1[:, P:2 * P],
                                     s_buf[:, t + 1, :],
                                     start=False, stop=True,
                                     skip_group_check=True)
                    nc.tensor.matmul(ps[:, 1, :], w1[:, P:2 * P],
                                     s_buf[:, t + 2, :],
                                     start=False, stop=True,
                                     skip_group_check=True)
                    if g == 0:
                        # edge tile: recip can exceed fp16 range where m==0
                        act_raw(nc, rt32[:], ps[:], Rec, bias=EPS)
                        nc.vector.tensor_mul(ratio[:, 0:2, :],
                                             mT[:, 0:2, :], rt32[:])
                    elif g % 2 == 0:
                        act_raw(nc, ratio[:, t:t + 2, :], ps[:], Rec, bias=EPS)
                        nc.gpsimd.tensor_mul(ratio[:, t:t + 2, :],
                                             ratio[:, t:t + 2, :],
                                             mT[:, t:t + 2, :])
                    else:
                        # DVE, split narrow: tile t first (conv2 group g-1
                        # waits on it), then t+1
                        act_raw(nc, ratio[:, t:t + 2, :], ps[:], Rec, bias=EPS)
                        nc.vector.tensor_mul(ratio[:, t, :], ratio[:, t, :],
                                             mT[:, t, :])
                        nc.vector.tensor_mul(ratio[:, t + 1, :],
                                             ratio[:, t + 1, :],
                                             mT[:, t + 1, :])

                def c2_update(it, j):
                    t = 2 * j
                    ps = pp2.tile([P, 2, BC], F32, tag="ps2")
                    nc.tensor.matmul(ps[:, 0, :], w2[:, 0:P], ratio[:, t, :],
                                     start=True, stop=False,
                                     skip_group_check=True)
                    nc.tensor.matmul(ps[:, 1, :], w2[:, 0:P],
                                     ratio[:, t + 1, :],
                                     start=True, stop=False,
                                     skip_group_check=True)
                    nc.tensor.matmul(ps[:, 0, :], w2[:, P:2 * P],
                                     ratio[:, t + 1, :],
                                     start=False, stop=True,
                                     skip_group_check=True)
                    nc.tensor.matmul(ps[:, 1, :], w2[:, P:2 * P],
                                     ratio[:, t + 2, :],
                                     start=False, stop=True,
                                     skip_group_check=True)
                    dst = s_buf[:, t + 1:t + 3, :]
                    if alpha_is_one:
                        nc.vector.scalar_tensor_tensor(
                            out=dst, in0=ps[:], scalar=EPS, in1=dst,
                            op0=mybir.AluOpType.add,
                            op1=mybir.AluOpType.mult)
                    else:
                        lg = auxp.tile([P, 2, BC], F32, tag="lg")
                        act_raw(nc, lg[:], ps[:], Ln, bias=EPS)
                        cp = auxp.tile([P, 2, BC], F16, tag="cp")
                        act_raw(nc, cp[:], lg[:], Exp,
                                scale=float(alpha64[it]))
                        nc.vector.tensor_mul(dst, dst, cp[:])

                for it in range(NITER):
                    c1_ratio(it, 0)
                    c1_ratio(it, 1)
                    c1_ratio(it, 2)
                    for j in range(NG2):
                        if j + 3 < NG1:
                            c1_ratio(it, j + 3)
                        c2_update(it, j)

            # ---- transpose back + cast fp32 + store ----
            # 8 chunks of 8 l-tiles, double-buffered; casts alternate DVE/ACT
            with tc.tile_pool(name="outp", bufs=2) as outp:
                for q in range(8):
                    sn16 = outp.tile([P, 32, P], F16, tag="sn16")
                    nc.sync.dma_start_transpose(
                        sn16[:], s_buf[:, 1 + q * 8:1 + (q + 1) * 8, :])
                    sn32 = outp.tile([P, 32, P], F32, tag="sn32")
                    if q % 2 == 0:
                        nc.vector.tensor_copy(sn32[:], sn16[:])
                    else:
                        act_raw(nc, sn32[:], sn16[:],
                                mybir.ActivationFunctionType.Copy)
                    sn32r = sn32.rearrange("p (tl bc) lp -> p tl bc lp",
                                           tl=8, bc=4)
                    for bc in range(4):
                        ydst = y_out[bc * P:(bc + 1) * P,
                                     q * 1024:(q + 1) * 1024].rearrange(
                            "p (tl lp) -> p tl lp", lp=P)
                        nc.sync.dma_start(ydst, sn32r[:, :, bc, :])

    split_multi_waits(nc)
    return nc


def kernel(m, psf, alpha):
    m = np.asarray(m)
    psf64 = np.asarray(psf, dtype=np.float64)
    alpha64 = np.asarray(alpha, dtype=np.float64)
    key = hashlib.sha256(
        psf64.tobytes() + alpha64.tobytes() + str(m.shape).encode()).hexdigest()
    if key not in _cache:
        _cache[key] = _build(psf64, alpha64)
    nc = _cache[key]

    from concourse.bass_utils import run_bass_kernel_spmd
    w1, w2 = _wpack(psf64)
    r0e, _ = _r0pack(psf64)
    in_maps = [{"m": np.ascontiguousarray(m[c * BC:(c + 1) * BC]).astype(np.float32),
                "w1": w1, "w2": w2, "r0": r0e} for c in range(N_CORES)]
    res = run_bass_kernel_spmd(nc, in_maps, core_ids=list(range(N_CORES)))
    out = np.concatenate([res.results[c]["y"] for c in range(N_CORES)], axis=0)
    return out.astype(np.float32)
